# revision 75
# baseline (speedup 1.0000x reference)
"""DeepseekMoE layer on 8 TRN2 NeuronCores — expert-parallel Bass/Tile kernel.

Strategy (self-contained, shapes hardcoded for this problem):
  H=2048, T=2048 tokens, E=16 experts, top-6, I=1408, shared IS=2816.

  Sharding (done on host inside kernel(), per the full-input contract):
    - Router (softmax + top-6) computed on host in fp32 (jax-on-CPU when
      available so near-tie selections match the jax reference bitwise)
      -> per-expert token lists (the "all-to-all dispatch" decision).
    - Core c owns experts 2c, 2c+1 (capacity-padded to CAP tokens each);
      shared expert sharded over its intermediate dim (352 rows per core,
      padded to 384 = 3*128).
    - Each core returns per-expert outputs [CAP, H] bf16 (pre-scaled by
      routing weights) and a dense shared partial [T, H] bf16; the host
      scatter-adds in fp32.

  Arithmetic: all matmuls run as fp8(e4m3) DoubleRow pairs at 0.5 cyc/row,
  using a hi+lo residual decomposition of every operand:
      a*s ~= a_hi + a_lo   (a_hi = fp8(a*s), a_lo = fp8(a*s - a_hi))
      a*b*s_a*s_b ~= a_hi*b_hi + a_lo*b_hi + a_hi*b_lo     (lo*lo dropped)
  Three DoubleRow instructions per k-tile pair = 0.75 cyc/row/k-tile, a
  1.33x speedup over bf16/fp32r with ~bf16 effective precision. The shared
  expert always runs the full 3-product form. Expert tokens are sorted by
  routing weight on the host so the first 128-token tile of each expert
  (the high-weight tokens, whose error is amplified by cw) keeps full
  precision, while the remaining tiles drop the w-residual products
  (hi-only w, split x/act) — their error is attenuated by small cw.
  Measured end-to-end rel err 1.17e-2 vs the 2e-2 gate.

  Scales: x*4, w1*64, w2*128. Stage-1 PSUM: gate = g*256 (Silu evicted with
  scale 2^-8); up = u*256, fused DVE (ps_u * 1/16) * silu_g = act*16, which
  is split hi/lo to fp8 for stage 2. Stage-2 PSUM = y*2048; eviction scale
  folds 2^-11 into the per-token routing weight (or a constant for the
  shared expert). Odd k-tile counts (11 expert, 3 shared) are handled by
  host-side slab rows: hi-slab gets a duplicated last k-tile, lo-slab gets
  zeros, plus a one-time memset of the act tile's pad k-tile.

  DMA (cost model: transfers serialize on a shared engine pool; each DMA
  also pays fixed issue overheads, and sub-512B contiguous runs halve
  bus efficiency): weights are host-packed so each o-tile's four w1 slabs
  load in ONE 2048B-contiguous-row DMA, w2 hi/lo slabs per h-chunk in one
  DMA each, and x hi/lo interleave in the innermost dim so a token-chunk
  is one DMA. w2 slabs prefetch right after each block's s1; later blocks'
  x loads stay at normal priority so they never delay the current stream.
"""

import os
import sys

sys.path.insert(0, "/opt/trn_rl_repo")

import numpy as np
import ml_dtypes

import concourse.bass as bass  # noqa: F401
import concourse.tile as tile
from concourse import bacc, mybir
from concourse.bass_utils import run_bass_kernel_spmd

H = 2048
T = 2048
E = 16
TOPK = 6
I2 = 2816  # 2*I
I = 1408
ISH = 2816  # shared intermediate (per gate/up half)
NCORES = 8
CAP0 = 832  # per-expert token capacity; grown in 64s if exceeded
SSL = 352  # shared-intermediate slice per core
SSLP = 384  # padded to 3*128

SX, SW1, SW2, SACT = 4.0, 64.0, 128.0, 16.0
S1_EVICT = 1.0 / (SW1 * SX)  # 2^-8
S1_UP = SACT / (SW1 * SX)  # 1/16
S2_EVICT = 1.0 / (SACT * SW2)  # 2^-11

F8NP = ml_dtypes.float8_e4m3
BF16NP = ml_dtypes.bfloat16
F32 = mybir.dt.float32
F8 = mybir.dt.float8e4
BF16 = mybir.dt.bfloat16
AF = mybir.ActivationFunctionType
ALU = mybir.AluOpType
DR = mybir.MatmulPerfMode.DoubleRow

_compiled = {}
last_result = None  # BassKernelResults of the most recent run (for profiling)


def _nchunks(n, first=None):
    """Split n into <=512 free-dim chunks; optional smaller first chunk so the
    first PSUM group starts after a fraction of the x block has landed."""
    out = [first] if first else []
    n -= first or 0
    while n > 0:
        w = min(512, n)
        out.append(w)
        n -= w
    return out


def _emit_mm3(nc, ps, wh_slab, wl_slab, x_tile, xc0, w, n_kt, first, last,
              use_wlo=True):
    """Accumulate sum_k w~[k].T @ x~[k] into ps[:, :w] via 3-product fp8
    DoubleRow instructions. wh/wl slabs: [128, n_kt(+pad), 128]; x_tile:
    [128, n_kt, 2, tokens] (hi/lo interleaved), token cols [xc0, xc0+w).
    n_kt may be odd: slabs carry a dup/zero pad k-tile (see module doc).
    use_wlo=False drops the w-residual product (2-product mode)."""
    xs = x_tile[:, :, xc0:xc0 + w, :]  # [128, kt, w, 2(hi/lo)]
    npair = n_kt // 2
    per = 3 if use_wlo else 2
    n3 = npair * per + (2 if n_kt % 2 else 0)
    i = 0
    for kp in range(npair):
        k = 2 * kp
        prods = [
            (wh_slab[:, k:k + 2, :], xs[:, k:k + 2, :, 0]),
            (wh_slab[:, k:k + 2, :], xs[:, k:k + 2, :, 1]),
        ]
        if use_wlo:
            prods.append((wl_slab[:, k:k + 2, :], xs[:, k:k + 2, :, 0]))
        for lhsT, rhs in prods:
            nc.tensor.matmul(ps[:, :w], lhsT, rhs,
                             start=(first and i == 0),
                             stop=(last and i == n3 - 1), perf_mode=DR)
            i += 1
    if n_kt % 2:
        k = n_kt - 1
        # (w_hi[k], w_hi[k]-dup) x (x_hi[k], x_lo[k]): slot dim from hi/lo
        nc.tensor.matmul(ps[:, :w], wh_slab[:, k:k + 2, :],
                         xs[:, k, :, :].rearrange("p w s -> p s w"),
                         start=(first and i == 0), stop=False, perf_mode=DR)
        # (w_lo[k], zeros) x (x_hi[k], x_hi[k+1]-garbage*0)
        nc.tensor.matmul(ps[:, :w], wl_slab[:, k:k + 2, :],
                         xs[:, k:k + 2, :, 0],
                         start=False, stop=last, perf_mode=DR)


def _emit_s1(nc, pools, *, b, x_tile, act_tile, chunks, ogroup=None):
    """Stage 1: per gate-o-tile, compute ps_g/ps_u via 3-product DoubleRow,
    then evict: ACT Silu -> ag; DVE (ps_u/16)*ag -> stage; ACT cast -> act_hi;
    DVE stage-hi -> act_lo. act_tile: [128, act_kt, 2, ntok] fp8.

    ogroup: if set, loop chunk-outer within o-groups of that size so the PE
    rides the incoming x stream instead of stalling o-by-o (startup block)."""
    w1p, psp, stp = pools["w1"], pools["ps"], pools["stage"]
    tc = pools["tc"]
    n_go = b["n_go"]
    spans = []
    t0 = 0
    for w in chunks:
        spans.append((t0, w))
        t0 += w

    hi_t = b.get("hi_t")  # tokens < hi_t: full 3-product; rest: hi-only w

    def load_slabs(o, hipri):
        # w1a rows o*512..o*512+512 hold the o-tile's 4 slabs (gwh, gwl,
        # uwh, uwl), each slab-major with its (k, o) run contiguous (2048B):
        # one DMA loads everything the o-tile needs
        t = w1p.tile([128, 4, 16, 128], F8, tag="w1slab",
                     name=f"w1_{b['tag']}_{o}")
        src = b["w1a"][o * 512:(o + 1) * 512, :].rearrange(
            "(s p) c -> p s c", p=128)
        if hipri:
            with tc.high_priority():  # gate/up halves land independently
                nc.sync.dma_start(out=t[:, 0:2], in_=src[:, 0:2])
                nc.sync.dma_start(out=t[:, 2:4], in_=src[:, 2:4])
        else:
            nc.sync.dma_start(out=t[:], in_=src)
        return [t[:, 0], t[:, 1], t[:, 2], t[:, 3]]

    def emit_o_chunk(o, slabs, ci, t0, w):
        gwh, gwl, uwh, uwl = slabs
        full = hi_t is None or t0 < hi_t
        ps_g = psp.tile([128, 512], F32, tag="ps", name=f"psg_{o}_{ci}")
        ps_u = psp.tile([128, 512], F32, tag="ps", name=f"psu_{o}_{ci}")
        _emit_mm3(nc, ps_g, gwh, gwl, x_tile, t0, w, 16, True, True,
                  use_wlo=full)
        _emit_mm3(nc, ps_u, uwh, uwl, x_tile, t0, w, 16, True, True,
                  use_wlo=full)
        ag = stp.tile([128, 512], F32, tag="stage", name=f"ag_{o}_{ci}")
        st = stp.tile([128, 512], F32, tag="stage", name=f"st_{o}_{ci}")
        nc.scalar.activation(out=ag[:, :w], in_=ps_g[:, :w], func=AF.Silu,
                             scale=S1_EVICT)
        nc.vector.scalar_tensor_tensor(
            out=st[:, :w], in0=ps_u[:, :w], scalar=S1_UP, in1=ag[:, :w],
            op0=ALU.mult, op1=ALU.mult)
        hi = act_tile[:, o, 0, t0:t0 + w]
        nc.scalar.activation(out=hi, in_=st[:, :w], func=AF.Copy)
        if full:  # act_lo only consumed by the full-precision s2 tiles
            nc.vector.tensor_sub(act_tile[:, o, 1, t0:t0 + w], st[:, :w], hi)

    if ogroup is None:
        for o in range(n_go):
            slabs = load_slabs(o, o <= 2 and b.get("hipri_slab"))
            for ci, (t0, w) in enumerate(spans):
                emit_o_chunk(o, slabs, ci, t0, w)
    else:
        for g0 in range(0, n_go, ogroup):
            os_ = range(g0, min(g0 + ogroup, n_go))
            slabs = {o: load_slabs(o, g0 == 0 and b.get("hipri_slab"))
                     for o in os_}
            for ci, (t0, w) in enumerate(spans):
                for o in os_:
                    emit_o_chunk(o, slabs[o], ci, t0, w)


def _prefetch_w2(nc, pools, b):
    """Load all of a block's w2 slabs (4 hc, hi+lo packed in one DMA each);
    emitted right after the block's s1 so they don't queue behind the next
    block's x transfers."""
    w2p = pools["w2"]
    kt = b["act_kt"]
    w2a_r = b["w2a"].rearrange("(s k p) h -> p s k h", s=2, p=128)
    res = b["w2_res"] if b.get("w2_res") is not None else b.setdefault(
        "w2_cache", [None] * 4)
    for hc in range(4):
        if res[hc] is not None:
            continue
        tag, bufs = ("w2res", 8) if b.get("w2_res") is not None else \
            ("w2slab", 6)
        sh = w2p.tile([128, kt, 512], F8, tag=tag, bufs=bufs,
                      name=f"w2h_{b['tag']}_{hc}")
        sl = w2p.tile([128, kt, 512], F8, tag=tag, bufs=bufs,
                      name=f"w2l_{b['tag']}_{hc}")
        nc.sync.dma_start(out=sh[:],
                          in_=w2a_r[:, 0, :, hc * 512:(hc + 1) * 512])
        nc.sync.dma_start(out=sl[:],
                          in_=w2a_r[:, 1, :, hc * 512:(hc + 1) * 512])
        res[hc] = (sh, sl)
    return res


def _emit_s2(nc, pools, *, b, act_tile, part):
    """Stage 2: out[t, hc] = sum_k act~[k].T @ w2~[k], 3-product DoubleRow
    with act (hi/lo) stationary and w2 slabs moving. Evict with per-token
    (expert) or constant (shared) scale to bf16, DMA out."""
    psp, outp = pools["ps2"], pools["out"]
    n_kt = b["act_kt"] - 1 if b["odd_kt"] else b["act_kt"]
    ntok = b["ntok"]
    hi_t = b.get("hi_t")
    slabs = _prefetch_w2(nc, pools, b)
    ntt = (ntok + 127) // 128
    nt1 = (ntt + 1) // 2
    tt_list = {1: range(nt1), 2: range(nt1, ntt)}[part]
    for tt in tt_list:
        r0 = tt * 128
        w = min(128, ntok - r0)
        full = hi_t is None or r0 < hi_t
        ysb = outp.tile([128, 2048], BF16, tag="ysb",
                        name=f"ysb_{b['tag']}_{tt}")
        for hc in range(4):
            w2h_slab, w2l_slab = slabs[hc]
            ps = psp.tile([128, 512], F32, tag="ps2", name=f"ps2_{hc}_{tt}")
            npair = n_kt // 2
            per = 3 if full else 1
            n3 = npair * per + (n_kt % 2) * (2 if full else 1)
            i = 0
            for kp in range(npair):
                k = 2 * kp
                prods = [
                    (act_tile[:, k:k + 2, 0, r0:r0 + w], w2h_slab[:, k:k + 2, :]),
                ]
                if full:
                    prods.append((act_tile[:, k:k + 2, 1, r0:r0 + w],
                                  w2h_slab[:, k:k + 2, :]))
                    prods.append((act_tile[:, k:k + 2, 0, r0:r0 + w],
                                  w2l_slab[:, k:k + 2, :]))
                for lhsT, rhs in prods:
                    nc.tensor.matmul(ps[:w, :], lhsT, rhs, start=(i == 0),
                                     stop=(i == n3 - 1), perf_mode=DR)
                    i += 1
            if n_kt % 2:
                k = n_kt - 1
                if full:
                    # (act_hi[k], act_lo[k]) x (w2h[k], w2h[k]-dup)
                    nc.tensor.matmul(ps[:w, :], act_tile[:, k, :, r0:r0 + w],
                                     w2h_slab[:, k:k + 2, :], start=False,
                                     stop=False, perf_mode=DR)
                    # (act_hi[k], pad-0) x (w2l[k], 0)
                    nc.tensor.matmul(ps[:w, :],
                                     act_tile[:, k:k + 2, 0, r0:r0 + w],
                                     w2l_slab[:, k:k + 2, :],
                                     start=False, stop=True, perf_mode=DR)
                else:
                    # (act_hi[k], pad-0) x (w2h[k], w2h[k]-dup): pad slot is 0
                    nc.tensor.matmul(ps[:w, :],
                                     act_tile[:, k:k + 2, 0, r0:r0 + w],
                                     w2h_slab[:, k:k + 2, :],
                                     start=False, stop=True, perf_mode=DR)
            # alternate evict engine so PSUM drain never paces the PE
            dst = ysb[:w, hc * 512:(hc + 1) * 512]
            if b["cw"] is not None:
                if hc % 2 == 0:
                    nc.scalar.activation(out=dst, in_=ps[:w, :], func=AF.Copy,
                                         scale=b["cw"][:w, tt:tt + 1])
                else:
                    nc.vector.tensor_scalar_mul(dst, ps[:w, :],
                                                b["cw"][:w, tt:tt + 1])
            else:
                if hc % 2 == 0:
                    nc.scalar.activation(out=dst, in_=ps[:w, :], func=AF.Copy,
                                         scale=S2_EVICT)
                else:
                    nc.vector.tensor_scalar_mul(dst, ps[:w, :], S2_EVICT)
        nc.sync.dma_start(
            out=b["out"][b["row0"] + r0: b["row0"] + r0 + w, :],
            in_=ysb[:w, :])


def _build(cap, order=(0, 1, 2, 3), first_chunk=256, defer_parts=(2,),
           ogroup0=None, w1bufs=3, hi_tiles=1):
    nc = bacc.Bacc("TRN2", target_bir_lowering=False, debug=False)

    cwcols = (cap + 127) // 128
    aps = {}
    for j in range(2):
        aps[f"xs{j}"] = nc.dram_tensor(f"xs{j}", [H, cap, 2], F8,
                                       kind="ExternalInput").ap()
        aps[f"w1a{j}"] = nc.dram_tensor(f"w1a{j}", [2 * I2, H], F8,
                                        kind="ExternalInput").ap()
        aps[f"w2a{j}"] = nc.dram_tensor(f"w2a{j}", [3072, H], F8,
                                        kind="ExternalInput").ap()
        aps[f"cw{j}"] = nc.dram_tensor(f"cw{j}", [cwcols * 128], F32,
                                       kind="ExternalInput").ap()
        aps[f"y{j}"] = nc.dram_tensor(f"y{j}", [cap, H], BF16,
                                      kind="ExternalOutput").ap()
    aps["xt"] = nc.dram_tensor("xt", [H, T, 2], F8, kind="ExternalInput").ap()
    aps["sw1a"] = nc.dram_tensor("sw1a", [4 * SSLP, H], F8,
                                 kind="ExternalInput").ap()
    aps["sw2a"] = nc.dram_tensor("sw2a", [1024, H], F8,
                                 kind="ExternalInput").ap()
    aps["ys"] = nc.dram_tensor("ys", [T, H], BF16, kind="ExternalOutput").ap()

    import contextlib
    with tile.TileContext(nc) as tc, contextlib.ExitStack() as ctx:
        pools = {
            "x": ctx.enter_context(tc.tile_pool(name="x", bufs=2)),
            "w1": ctx.enter_context(tc.tile_pool(name="w1", bufs=w1bufs)),
            "w2": ctx.enter_context(tc.tile_pool(name="w2", bufs=4)),
            "act": ctx.enter_context(tc.tile_pool(name="act", bufs=2)),
            "stage": ctx.enter_context(tc.tile_pool(name="stage", bufs=4)),
            "out": ctx.enter_context(tc.tile_pool(name="out", bufs=3)),
            # separate s1/s2 PSUM pools: the cross-block s2 deferral must
            # never be starved of PSUM slots by the next block's stalled s1
            "ps": ctx.enter_context(tc.tile_pool(name="ps", bufs=4,
                                                 space="PSUM")),
            "ps2": ctx.enter_context(tc.tile_pool(name="ps2", bufs=4,
                                                  space="PSUM")),
            "misc": ctx.enter_context(tc.tile_pool(name="misc", bufs=2)),
        }
        pools["tc"] = tc
        cw_tiles = {}

        def get_cw(j):
            if j not in cw_tiles:
                cw_r = aps[f"cw{j}"].rearrange("(n p) -> p n", p=128)
                cw_tiles[j] = pools["misc"].tile([128, cwcols], F32,
                                                 tag=f"cw{j}", name=f"cw{j}_t")
                nc.sync.dma_start(out=cw_tiles[j][:], in_=cw_r[:])
            return cw_tiles[j]



        shared_res = [None, None, None, None]
        all_blocks = []
        for j in range(2):
            all_blocks.append(dict(
                tag=f"e{j}", n_go=11, act_kt=12, odd_kt=True, ntok=cap,
                x_ap=aps[f"xs{j}"], x_off=0,
                w1a=aps[f"w1a{j}"], w2a=aps[f"w2a{j}"],
                out=aps[f"y{j}"], row0=0, cw_j=j, w2_res=None,
                hipri_slab=True,
                hi_t=None if hi_tiles is None else 128 * hi_tiles,
            ))
        for half in range(2):
            all_blocks.append(dict(
                tag=f"sh{half}", n_go=3, act_kt=4, odd_kt=True, ntok=1024,
                x_ap=aps["xt"], x_off=half * 1024,
                w1a=aps["sw1a"], w2a=aps["sw2a"],
                out=aps["ys"], row0=half * 1024, cw_j=None,
                w2_res=shared_res, hipri_slab=True,
            ))

        blocks = [all_blocks[i] for i in order]

        def s1_chunks(n):
            b = blocks[n]
            hi_t = b.get("hi_t")
            if hi_t:  # chunk boundary must align with the precision boundary
                rem = b["ntok"] - hi_t
                if rem > 512:  # near-equal halves keep the x-stream wait low
                    first = min(512, (rem // 2 + 63) // 64 * 64)
                    return [hi_t] + _nchunks(rem, first=first)
                return [hi_t] + _nchunks(rem)
            return _nchunks(b["ntok"], first=first_chunk if n == 0 else None)

        def load_x(n, first_hipri=False):
            # only block 0's first chunk is urgent; later blocks' x loads are
            # prefetches that must NOT outrank the current block's stream
            b = blocks[n]
            x_r = b["x_ap"].rearrange("(k p) t s -> p k t s", p=128)
            xt_tile = pools["x"].tile([128, 16, b["ntok"], 2], F8, tag="xsel",
                                      name=f"x_{b['tag']}")
            t0 = 0
            for ci, w in enumerate(s1_chunks(n)):
                src = x_r[:, :, b["x_off"] + t0: b["x_off"] + t0 + w, :]
                dst = xt_tile[:, :, t0:t0 + w, :]
                if ci == 0 and first_hipri:
                    with tc.high_priority():
                        nc.sync.dma_start(out=dst, in_=src)
                else:
                    nc.sync.dma_start(out=dst, in_=src)
                t0 += w
            return xt_tile

        def emit_s2_part(b, act_tile, part):
            if "cw" not in b:
                b["cw"] = None if b["cw_j"] is None else get_cw(b["cw_j"])
            _emit_s2(nc, pools, b=b, act_tile=act_tile, part=part)

        # Emit s1(n), then block n+1's x-load, then the previous block's
        # deferred s2 half, then s2(n) part 1: the next x-load overlaps s2(n)
        # compute instead of queueing behind its weight slabs.
        x_tiles = [load_x(0, first_hipri=True)]
        get_cw(0)  # small; emitted after block 0's x so they don't delay it
        get_cw(1)
        deferred = None
        for n, b in enumerate(blocks):
            act_tile = pools["act"].tile([128, b["act_kt"], 2, b["ntok"]], F8,
                                         tag="act", name=f"act_{b['tag']}")
            # zero the pad k-tile (hi slot is read by the odd-k leftover
            # instruction; lo slot never read)
            nc.gpsimd.memset(act_tile[:, b["act_kt"] - 1, 0, :], 0.0)
            _emit_s1(nc, pools, b=b, x_tile=x_tiles[n], act_tile=act_tile,
                     chunks=s1_chunks(n), ogroup=ogroup0 if n == 0 else None)
            _prefetch_w2(nc, pools, b)
            if n + 1 < len(blocks):
                x_tiles.append(load_x(n + 1))
            if deferred is not None:
                for p in defer_parts:
                    emit_s2_part(*deferred, part=p)
                deferred = None
            for p in (1, 2):
                if p not in defer_parts:
                    emit_s2_part(b, act_tile, part=p)
            deferred = (b, act_tile)
        if deferred is not None:
            for p in defer_parts:
                emit_s2_part(*deferred, part=p)

    nc.compile()
    return nc


def _route(xf, gate_w):
    """Host router: fp32 softmax + top-6, matching jax bitwise when possible."""
    try:
        import jax
        import jax.numpy as jnp

        cpu = jax.devices("cpu")[0]
        with jax.default_device(cpu):
            logits = jnp.asarray(xf) @ jnp.asarray(gate_w).T
            probs = jax.nn.softmax(logits.astype(jnp.float32), axis=-1)
            _, sel = jax.lax.top_k(probs, TOPK)
        return np.asarray(probs), np.asarray(sel)
    except Exception:
        logits = xf @ gate_w.T
        m = logits.max(axis=-1, keepdims=True)
        e = np.exp(logits - m, dtype=np.float32)
        probs = e / e.sum(axis=-1, keepdims=True)
        sel = np.argsort(-probs, axis=-1, kind="stable")[:, :TOPK]
        return probs, sel


def _split8(a, s):
    """-> (hi, lo) fp8 arrays with a*s ~= hi + lo."""
    sa = (a * s).astype(np.float32)
    hi = sa.astype(F8NP)
    lo = (sa - hi.astype(np.float32)).astype(F8NP)
    return hi, lo


def _slab_major(w):
    """[H, O] -> [O, H] slab-major: row ot*128+p holds slab ot's (k, o) run
    contiguously, so each w1 slab DMA moves 2048B-contiguous rows."""
    Hd, O = w.shape
    return np.ascontiguousarray(
        w.reshape(Hd // 128, 128, O // 128, 128).transpose(2, 1, 0, 3)
        .reshape(O, Hd))


def kernel(x, gate_w, w1, w2, shared_w1, shared_w2):
    x = np.asarray(x, np.float32)
    gate_w = np.asarray(gate_w, np.float32)
    w1 = np.asarray(w1, np.float32)
    w2 = np.asarray(w2, np.float32)
    shared_w1 = np.asarray(shared_w1, np.float32)
    shared_w2 = np.asarray(shared_w2, np.float32)

    B, S, Hd = x.shape
    xf = np.ascontiguousarray(x.reshape(-1, Hd))  # [T, H]

    probs, sel = _route(xf, gate_w)
    onehot = np.zeros((T, E), bool)
    onehot[np.arange(T)[:, None], sel] = True
    # sort each expert's tokens by routing weight (descending) so the first
    # token tiles hold the high-weight tokens that get full 3-product
    # precision; low-weight tokens use the cheap hi-only products
    idx_e = []
    for e in range(E):
        ix = np.nonzero(onehot[:, e])[0]
        idx_e.append(ix[np.argsort(-probs[ix, e], kind="stable")])
    counts = np.array([len(ix) for ix in idx_e])

    cap = CAP0
    while counts.max() > cap:
        cap += 64
    if cap not in _compiled:
        _compiled[cap] = _build(cap, order=(0, 1, 2, 3), defer_parts=(),
                                hi_tiles=1)
    nc = _compiled[cap]

    # quantize x once: [T, H] hi/lo, packed [H, T, 2]
    xq_hi, xq_lo = _split8(xf, SX)
    xt2 = np.empty((H, T, 2), F8NP)
    xt2[:, :, 0] = xq_hi.T
    xt2[:, :, 1] = xq_lo.T

    def pack_w1(hi, lo, n_go):
        """[H, 2*n_go*128] hi/lo -> [n_go, 4, 128, H] o-tile groups
        (gwh, gwl, uwh, uwl), each slab-major."""
        hs = _slab_major(hi).reshape(2 * n_go, 128, H)
        ls = _slab_major(lo).reshape(2 * n_go, 128, H)
        out = np.empty((n_go, 4, 128, H), F8NP)
        out[:, 0] = hs[:n_go]
        out[:, 1] = ls[:n_go]
        out[:, 2] = hs[n_go:]
        out[:, 3] = ls[n_go:]
        return out.reshape(4 * n_go * 128, H)

    cwcols = (cap + 127) // 128
    in_maps = []
    for c in range(NCORES):
        m = {"xt": xt2}
        for j in range(2):
            e = 2 * c + j
            ix = idx_e[e]
            xs2 = np.zeros((H, cap, 2), F8NP)
            xs2[:, :len(ix), 0] = xq_hi[ix].T
            xs2[:, :len(ix), 1] = xq_lo[ix].T
            m[f"xs{j}"] = xs2
            hi, lo = _split8(w1[e].T, SW1)  # [H, I2]
            m[f"w1a{j}"] = pack_w1(hi, lo, 11)
            hi, lo = _split8(w2[e].T, SW2)  # [I, H]
            w2a = np.zeros((3072, H), F8NP)
            w2a[:I] = hi
            w2a[I:1536] = hi[-128:]  # dup of k-tile 10 for the odd-k leftover
            w2a[1536:1536 + I] = lo
            m[f"w2a{j}"] = w2a
            cw = np.zeros(cwcols * 128, np.float32)
            cw[: len(ix)] = probs[ix, e] * S2_EVICT
            m[f"cw{j}"] = cw
        sl = slice(SSL * c, SSL * (c + 1))
        sg = np.zeros((H, SSLP), np.float32)
        su = np.zeros((H, SSLP), np.float32)
        sg[:, :SSL] = shared_w1[sl].T
        su[:, :SSL] = shared_w1[ISH + SSL * c: ISH + SSL * (c + 1)].T
        hi_g, lo_g = _split8(sg, SW1)
        hi_u, lo_u = _split8(su, SW1)
        m["sw1a"] = pack_w1(np.concatenate([hi_g, hi_u], axis=1),
                            np.concatenate([lo_g, lo_u], axis=1), 3)
        s2w = np.zeros((512, H), np.float32)
        s2w[:SSL] = shared_w2[:, sl].T
        hi, lo = _split8(s2w, SW2)
        hi[SSLP:] = hi[SSLP - 128: SSLP]  # dup k-tile 2
        lo[SSLP:] = 0
        m["sw2a"] = np.concatenate([hi, lo], axis=0)
        in_maps.append(m)

    try:
        res = run_bass_kernel_spmd(nc, in_maps, list(range(NCORES)))
    except ModuleNotFoundError:
        os.environ["BASS_NEVER_TRACE"] = "1"
        res = run_bass_kernel_spmd(nc, in_maps, list(range(NCORES)))
    global last_result
    last_result = res

    out = np.zeros((T, H), np.float32)
    for c in range(NCORES):
        out += res.results[c]["ys"].astype(np.float32)
        for j in range(2):
            e = 2 * c + j
            ix = idx_e[e]
            out[ix] += res.results[c][f"y{j}"][: len(ix)].astype(np.float32)

    return out.reshape(B, S, Hd)
